# revision 40
# baseline (speedup 1.0000x reference)
"""Self-contained Trainium2 Bass kernel for nn_Attention_59253368816224.

GQA attention block: per-head RMSNorm on Q/K, RoPE, causal softmax
attention, o_proj.  B=2, S=2048, H=2048, 16 Q heads / 4 KV heads,
head_dim=128.

Sharding: 8 cores = 2 batches x 4 KV groups.  Core c -> (b=c//4, g=c%4)
owns 4 Q heads + 1 KV head.  o_proj is row-parallel: each core emits a
partial [S, H] output contracted over its 512 hidden dims; the host sums
the 4 partials per batch in fp32.

fp8 (e4m3) DoubleRow (2x bf16 throughput on 256-deep contractions) is
used ONLY where its ~3.6% per-element quantization noise is attenuated
by softmax averaging (verified vs a host emulation of the full
quantization pipeline):
  - q/k projection (k-tile pairs): noise enters scores only as exp
    jitter, averaged over the effective key count.
  - v projection for seq>=512 and attn@v for query chunks ic>=1: short
    rows (few keys, no averaging) keep an exact bf16 path; long rows
    attenuate.
  - rowsum over off-diagonal key-tile pairs (fp8 ones).
Everything on the direct output path (o_proj, otsb, Wo) stays bf16:
matmul-input noise passes to the output UNattenuated (random-sign
contraction preserves SNR), and a single e4m3 stage there costs ~2.6%
max-rel, more than the whole 2e-2 budget.

All fp8 operands are pre-scaled by powers of two into e4m3's normal
range (subnormals flush on HW): x*8, Wq/Wk/Wv*256 (RMSNorm cancels the
q/k scale; the ic=0 rowsum uses an ALPHA-valued ones matrix to cancel
the v scale), exp biased by -ln4 (max observed score ~5.3 -> exp/4 ~ 49
< e4m3 max).

Device pipeline:
  A) W-stationary QKV projection from host-pre-transposed fp8 xT,
     DoubleRow over k-tile pairs (v cols [0,512) from a parallel bf16
     copy of x), producing transposed qT/kT/vT [d, s]; per-column
     sum-of-squares via selector matmuls -> RMSNorm scale via ACT
     Abs_reciprocal_sqrt (same table set as Square/Copy; frees DVE).
     Scale/rope tails are emitted one proj group late so their sst/rot
     matmuls never stall the in-order tensor queue.
  B) RoPE in the transposed domain: rot(q) via a constant 128x128
     permutation matmul; combine with w-folded cosT/sinT tables.
  C) v transposed back to natural [s, d] with a DMA transpose (bf16,
     ALPHA-scaled), then cast to fp8 v_true with a 2^-11 rescale.
  D) Flash-style causal attention on transposed scores [j, i]: key
     tiles processed in pairs sharing a 2-bank PSUM tile, one exp per
     off-diagonal pair; 128-col triangle masks on DVE.  Chunk ic=0 runs
     fully bf16; chunks ic>=1 write fp8 at tiles, attn@v DoubleRow per
     off-diagonal pair, rowsum DoubleRow per pair (no DVE pre-sums);
     normalizer via DVE fast reciprocal.
  E) o_proj bf16 from otsb against Wo rows (PSUM->SBUF casts split
     scalar/vector), partial output to DRAM bf16.
"""

import os
import sys
import numpy as np
import ml_dtypes

BF16 = ml_dtypes.bfloat16
FP8 = ml_dtypes.float8_e4m3

B = 2
S = 2048
H = 2048
NQH = 16          # total q heads
NKV = 4           # total kv heads
HD = 128          # head dim
GQ = 4            # q heads per core (per kv group)
KT = H // 128     # 16 k-tiles over hidden
ST = S // 128     # 16 s-tiles
RMS_EPS = 1.1920928955078125e-07
INV_SQRT_HD = 1.0 / float(np.sqrt(HD))
LN4 = float(np.log(4.0))

S_X = 8.0          # host scale on x before fp8 quantization
S_W = 256.0        # host scale on Wq/Wk/Wv
ALPHA = S_X * S_W  # scale carried by q/k/v out of the projection (2048)

_PROGRAM = None


def _build_program(shared_rope=True):
    import concourse.bacc as bacc
    import concourse.tile as tile
    from concourse import mybir
    from contextlib import ExitStack

    bf = mybir.dt.bfloat16
    f8 = mybir.dt.float8e4
    DR = mybir.MatmulPerfMode.DoubleRow

    nc = bacc.Bacc("TRN2", target_bir_lowering=False, debug=False, num_devices=8)

    # ---- DRAM I/O (per-core values supplied via in_maps) ----
    xt_d = nc.dram_tensor("xt", (KT, 128, S), f8, kind="ExternalInput")
    xtb_d = nc.dram_tensor("xtb", (KT, 128, 256), bf, kind="ExternalInput")
    # weights arrive pre-packed in SBUF layout [128, k, d]: a plain
    # contiguous DMA costs ~0.6us to issue vs ~3-5us for the gather
    # pattern of an on-the-fly (k p) d -> p k d rearrange
    wq_d = nc.dram_tensor("wq", (128, KT, GQ * HD), f8, kind="ExternalInput")
    wqb_d = nc.dram_tensor("wqb", (128, KT, GQ * HD), bf, kind="ExternalInput")
    wk_d = nc.dram_tensor("wk", (128, KT, HD), f8, kind="ExternalInput")
    wkb_d = nc.dram_tensor("wkb", (128, KT, HD), bf, kind="ExternalInput")
    wv_d = nc.dram_tensor("wv", (128, KT, HD), f8, kind="ExternalInput")
    wvb_d = nc.dram_tensor("wvb", (128, KT, HD), bf, kind="ExternalInput")
    wo_d = nc.dram_tensor("wo", (128, GQ, H), bf, kind="ExternalInput")
    cosq_d = nc.dram_tensor("cosq", (HD, S), bf, kind="ExternalInput")
    sinq_d = nc.dram_tensor("sinq", (HD, S), bf, kind="ExternalInput")
    if not shared_rope:
        cosk_d = nc.dram_tensor("cosk", (HD, S), bf, kind="ExternalInput")
        sink_d = nc.dram_tensor("sink", (HD, S), bf, kind="ExternalInput")
    rmat_d = nc.dram_tensor("rmat", (128, 128), bf, kind="ExternalInput")
    ones_d = nc.dram_tensor("ones", (128, 128), bf, kind="ExternalInput")
    onesa_d = nc.dram_tensor("onesa", (128, 128), bf, kind="ExternalInput")
    ones8_d = nc.dram_tensor("ones8", (128, 2, 128), f8, kind="ExternalInput")
    mask_d = nc.dram_tensor("mask", (128, 128), f8, kind="ExternalInput")
    maskb_d = nc.dram_tensor("maskb", (128, 128), bf, kind="ExternalInput")
    out_d = nc.dram_tensor("out", (S, H), bf, kind="ExternalOutput")
    # internal scratch for the v transpose
    vt_scratch = nc.dram_tensor("vt_scratch", (HD, S), bf)

    Exp = mybir.ActivationFunctionType.Exp
    ARsqrt = mybir.ActivationFunctionType.Abs_reciprocal_sqrt
    Square = mybir.ActivationFunctionType.Square

    with tile.TileContext(nc) as tc:
        with ExitStack() as ctx:
            consts = ctx.enter_context(tc.tile_pool(name="consts", bufs=1))
            persist = ctx.enter_context(tc.tile_pool(name="persist", bufs=1))

            # ---- persistent intermediates ----
            qkvbf = persist.tile([128, 5, S], bf)      # raw transposed q(4)/k
            vt = persist.tile([128, S], bf)            # raw transposed v
            qfin = persist.tile([128, GQ, S], bf)      # roped+normed qT
            kfin = persist.tile([128, S], bf)          # roped+normed kT
            v3 = persist.tile([128, ST, HD], bf)       # v natural (x ALPHA)
            v38 = persist.tile([128, ST, HD], f8)      # v natural fp8 (true)
            otsb = persist.tile([128, GQ, S], bf)      # oT per head
            wo_sb = persist.tile([128, GQ, H], bf)     # o_proj weights

            # ---- constant tiles ----
            cosq = consts.tile([128, S], bf)
            sinq = consts.tile([128, S], bf)
            if shared_rope:
                cosk, sink = cosq, sinq
            else:
                cosk = consts.tile([128, S], bf)
                sink = consts.tile([128, S], bf)
            rmat = consts.tile([128, 128], bf)
            onesm = consts.tile([128, 128], bf)
            onesa = consts.tile([128, 128], bf)
            ones8 = consts.tile([128, 2, 128], f8)
            masks = consts.tile([128, 128], f8)
            masksb = consts.tile([128, 128], bf)
            eps128 = consts.tile([128, 1], mybir.dt.float32)
            nc.vector.memset(eps128[:], RMS_EPS)
            nln4 = consts.tile([128, 1], mybir.dt.float32)
            nc.vector.memset(nln4[:], -LN4)

            # ============ Phase A: QKV projection (+ per-chunk scales) ====
            with tc.tile_pool(name="proj_in", bufs=1) as proj_in, \
                 tc.tile_pool(name="sqp", bufs=2) as sqp, \
                 tc.tile_pool(name="scb", bufs=4) as scbp, \
                 tc.tile_pool(name="ropet", bufs=2) as ropet:
                xtall = proj_in.tile([128, KT, S], f8, tag="xtall", name="xtall")
                xtb16 = proj_in.tile([128, KT, 256], bf, tag="xtb16", name="xtb16")
                wqall = proj_in.tile([128, KT, GQ * HD], f8, tag="wqall", name="wqall")
                wqb16 = proj_in.tile([128, KT, GQ * HD], bf, tag="wqb16", name="wqb16")
                wkall = proj_in.tile([128, KT, HD], f8, tag="wkall", name="wkall")
                wkb16 = proj_in.tile([128, KT, HD], bf, tag="wkb16", name="wkb16")
                wvall = proj_in.tile([128, KT, HD], f8, tag="wvall", name="wvall")
                wvb = proj_in.tile([128, KT, HD], bf, tag="wvb", name="wvb")
                # DMA issue costs ~0.6us per call on an engine's serial
                # queue: spread input issues across THREE queues (sync:
                # fp8 x + fp8 weights; vector: bf16 x + bf16 weights;
                # scalar: tables + o_proj weights) so transfers start early
                # and the q weights aren't stuck behind 16 x-tile issues.
                # the very first matmul (k-proj bf16 chain, k=0) needs only
                # wkb16 + xtb16[0]: put them first so compute starts ~10us
                # earlier; the fp8 DR operands follow immediately after
                nc.sync.dma_start(out=wkb16[:], in_=wkb_d[:])
                nc.sync.dma_start(out=xtb16[:, 0, :], in_=xtb_d[0, :, :])
                nc.sync.dma_start(out=xtb16[:, 1, :], in_=xtb_d[1, :, :])
                nc.sync.dma_start(out=wvb[:], in_=wvb_d[:])
                nc.sync.dma_start(out=wkall[:], in_=wk_d[:])
                nc.sync.dma_start(out=xtall[:, 0, :], in_=xt_d[0, :, :])
                nc.sync.dma_start(out=xtall[:, 1, :], in_=xt_d[1, :, :])
                nc.sync.dma_start(out=wvall[:], in_=wv_d[:])
                for k in range(2, KT):
                    nc.sync.dma_start(out=xtb16[:, k, :], in_=xtb_d[k, :, :])
                    nc.sync.dma_start(out=xtall[:, k, :], in_=xt_d[k, :, :])
                nc.sync.dma_start(out=wqall[:], in_=wq_d[:])
                nc.sync.dma_start(out=wqb16[:], in_=wqb_d[:])
                nc.scalar.dma_start(out=cosq[:], in_=cosq_d[:])
                nc.scalar.dma_start(out=sinq[:], in_=sinq_d[:])
                if not shared_rope:
                    nc.scalar.dma_start(out=cosk[:], in_=cosk_d[:])
                    nc.scalar.dma_start(out=sink[:], in_=sink_d[:])
                nc.scalar.dma_start(out=rmat[:], in_=rmat_d[:])
                nc.scalar.dma_start(out=onesm[:], in_=ones_d[:])
                nc.scalar.dma_start(out=onesa[:], in_=onesa_d[:])
                nc.scalar.dma_start(out=ones8[:], in_=ones8_d[:])
                nc.scalar.dma_start(out=masks[:], in_=mask_d[:])
                nc.scalar.dma_start(out=masksb[:], in_=maskb_d[:])
                # o_proj weights stream during the post-input DMA lull so
                # phase E doesn't wait on them
                nc.scalar.dma_start(out=wo_sb[:], in_=wo_d[:])

                scbs = {}

                def make_tail(c, half, ps):
                    # copy + norm-scale + rope for one finished proj group.
                    # Emitted AFTER the next group's matmuls so the sst/rot
                    # matmuls never stall the in-order tensor queue.
                    def tail():
                        h0 = half * 1024
                        dst = qkvbf[:, c, h0:h0 + 1024]
                        # scale = 1/sqrt(mean+eps) via Abs_reciprocal_sqrt
                        # (same ACT table set as Square/Copy).
                        # Square first: it reads PSUM directly, so the sst
                        # matmul isn't queued behind the copy on scalar
                        sq = sqp.tile([128, 1024], bf, tag="sq", name="sq")
                        nc.scalar.activation(sq[:], ps[:], Square)
                        nc.scalar.copy(dst, ps[:])
                        for nn in range(2):
                            sst = ss_ps.tile(
                                [128, 512], mybir.dt.float32,
                                tag="sst", name=f"sst_{c}_{half}_{nn}",
                            )
                            nc.tensor.matmul(
                                sst[:],
                                onesm[:],
                                sq[:, nn * 512:(nn + 1) * 512],
                                start=True,
                                stop=True,
                            )
                            scb = scbp.tile(
                                [128, 512], mybir.dt.float32,
                                tag="scb", name="scb",
                            )
                            scbs[(c, half * 2 + nn)] = scb
                            nc.scalar.activation(
                                scb[:], sst[:], ARsqrt,
                                bias=eps128[:], scale=1.0 / HD,
                            )
                        cosx = cosq if c < 4 else cosk
                        sinx = sinq if c < 4 else sink
                        for q4 in (2 * half, 2 * half + 1):
                            o0 = q4 * 512
                            scb = scbs[(c, q4)]
                            src_ap = qkvbf[:, c, o0:o0 + 512]
                            rot = rot_psp.tile(
                                [128, 512], mybir.dt.float32,
                                tag="rot", name="rot",
                            )
                            nc.tensor.matmul(
                                rot[:], rmat[:], src_ap, start=True, stop=True
                            )
                            a = ropet.tile([128, 512], bf, tag="a")
                            bb = ropet.tile([128, 512], bf, tag="b")
                            cc = ropet.tile([128, 512], bf, tag="c")
                            nc.gpsimd.tensor_mul(a[:], src_ap, cosx[:, o0:o0 + 512])
                            nc.vector.tensor_mul(bb[:], rot[:], sinx[:, o0:o0 + 512])
                            nc.vector.tensor_add(cc[:], a[:], bb[:])
                            dst2 = (
                                qfin[:, c, o0:o0 + 512]
                                if c < 4
                                else kfin[:, o0:o0 + 512]
                            )
                            nc.vector.tensor_mul(dst2, cc[:], scb[:])
                    return tail

                def make_late_tail(half, sq):
                    # norm-scale + rope for the k chunk, computed from the
                    # squared values after its PSUM pool has closed
                    def tail():
                        for nn in range(2):
                            sst = ss_ps.tile(
                                [128, 512], mybir.dt.float32,
                                tag="sst", name=f"sst_4_{half}_{nn}",
                            )
                            nc.tensor.matmul(
                                sst[:],
                                onesm[:],
                                sq[:, nn * 512:(nn + 1) * 512],
                                start=True,
                                stop=True,
                            )
                            scb = scbp.tile(
                                [128, 512], mybir.dt.float32,
                                tag="scb", name="scb",
                            )
                            scbs[(4, half * 2 + nn)] = scb
                            nc.scalar.activation(
                                scb[:], sst[:], ARsqrt,
                                bias=eps128[:], scale=1.0 / HD,
                            )
                        for q4 in (2 * half, 2 * half + 1):
                            o0 = q4 * 512
                            scb = scbs[(4, q4)]
                            src_ap = qkvbf[:, 4, o0:o0 + 512]
                            rot = rot_psp.tile(
                                [128, 512], mybir.dt.float32,
                                tag="rot", name="rot",
                            )
                            nc.tensor.matmul(
                                rot[:], rmat[:], src_ap, start=True, stop=True
                            )
                            a = ropet.tile([128, 512], bf, tag="a")
                            bb = ropet.tile([128, 512], bf, tag="b")
                            cc = ropet.tile([128, 512], bf, tag="c")
                            nc.gpsimd.tensor_mul(a[:], src_ap, cosk[:, o0:o0 + 512])
                            nc.vector.tensor_mul(bb[:], rot[:], sink[:, o0:o0 + 512])
                            nc.vector.tensor_add(cc[:], a[:], bb[:])
                            nc.vector.tensor_mul(
                                kfin[:, o0:o0 + 512], cc[:], scb[:]
                            )
                    return tail

                # ---- k and v projections jointly while x tiles stream in.
                # k: DoubleRow fp8 over k-tile pairs.  v: bf16 for seq cols
                # [0,512) (exact path for short attention rows), DoubleRow
                # fp8 beyond.  Their 4 accumulator groups take all 8 PSUM
                # banks, so this pool closes before the ss/rot pools open.
                sq4 = []
                with tc.tile_pool(name="ps45", bufs=4, space="PSUM") as ps45p:
                    ps45 = [
                        ps45p.tile(
                            [128, 1024], mybir.dt.float32,
                            tag="ps45", name=f"ps45_{g}",
                        )
                        for g in range(4)
                    ]
                    for j in range(KT // 2):
                        # k proj seq [0,256): bf16 (exact path for short
                        # attention rows), two k-tiles per j
                        for s_ in range(2):
                            k = 2 * j + s_
                            nc.tensor.matmul(
                                ps45[0][:, 0:256],
                                wkb16[:, k, :],
                                xtb16[:, k, :],
                                start=(k == 0),
                                stop=(k == KT - 1),
                                skip_group_check=True,
                            )
                        # k proj seq [256,2048): DR pairs (ragged first)
                        for (g, o0, o1) in ((0, 256, 512), (0, 512, 1024),
                                            (1, 1024, 1536), (1, 1536, 2048)):
                            # the [256:512] region shares a PSUM bank with
                            # the bf16 chain: its k==0 start already zeroed
                            # the whole 2KB zero-region, so never re-start
                            nc.tensor.matmul(
                                ps45[g][:, o0 - g * 1024:o1 - g * 1024],
                                wkall[:, 2 * j:2 * j + 2, :],
                                xtall[:, 2 * j:2 * j + 2, o0:o1],
                                start=(j == 0 and o0 != 256),
                                stop=(j == KT // 2 - 1),
                                perf_mode=DR,
                                skip_group_check=True,
                            )
                        # v proj seq [0,256): bf16, two k-tiles per j
                        for s_ in range(2):
                            k = 2 * j + s_
                            nc.tensor.matmul(
                                ps45[2][:, 0:256],
                                wvb[:, k, :],
                                xtb16[:, k, :],
                                start=(k == 0),
                                stop=(k == KT - 1),
                                skip_group_check=True,
                            )
                        # v proj seq [256,2048): DR pairs (ragged first)
                        for (g, o0, o1) in ((2, 256, 512), (2, 512, 1024),
                                            (3, 1024, 1536), (3, 1536, 2048)):
                            nc.tensor.matmul(
                                ps45[g][:, o0 - (g - 2) * 1024:o1 - (g - 2) * 1024],
                                wvall[:, 2 * j:2 * j + 2, :],
                                xtall[:, 2 * j:2 * j + 2, o0:o1],
                                start=(j == 0 and o0 != 256),
                                stop=(j == KT // 2 - 1),
                                perf_mode=DR,
                                skip_group_check=True,
                            )
                    for half in range(2):
                        sq = sqp.tile([128, 1024], bf, tag="sq", name="sq4")
                        nc.scalar.activation(sq[:], ps45[half][:], Square)
                        nc.scalar.copy(
                            qkvbf[:, 4, half * 1024:(half + 1) * 1024],
                            ps45[half][:],
                        )
                        sq4.append(sq)
                        nc.vector.tensor_copy(
                            vt[:, half * 1024:(half + 1) * 1024],
                            ps45[2 + half][:],
                        )
                    # v transpose DMAs issue on the sync queue, which has
                    # drained its input issues by this point
                    nc.sync.dma_start(out=vt_scratch[:], in_=vt[:])
                    nc.sync.dma_start_transpose(out=v3[:], in_=vt_scratch[:])
                    # fp8 cast removing the ALPHA scale (exact power of 2)
                    nc.vector.tensor_scalar_mul(v38[:], v3[:], 1.0 / ALPHA)

                with tc.tile_pool(name="qkv_ps", bufs=2, space="PSUM") as qkv_ps, \
                     tc.tile_pool(name="ss_ps", bufs=2, space="PSUM") as ss_ps, \
                     tc.tile_pool(name="rot_ps", bufs=2, space="PSUM") as rot_psp:
                    tails = [make_late_tail(0, sq4[0]), make_late_tail(1, sq4[1])]
                    # half=1 groups first: they are pure DoubleRow and need
                    # only wqall, covering the in-flight wqb16 transfer with
                    # ~13us of matmuls before the bf16 chains start
                    for (c, half) in ((0, 1), (1, 1), (2, 1), (3, 1),
                                      (0, 0), (1, 0), (2, 0), (3, 0)):
                        if True:
                            h0 = half * 1024
                            ps = qkv_ps.tile(
                                [128, 1024], mybir.dt.float32, tag="ps", name="ps"
                            )
                            if half == 0:
                                # q proj seq [0,256): bf16 exact path
                                for k in range(KT):
                                    nc.tensor.matmul(
                                        ps[:, 0:256],
                                        wqb16[:, k, c * 128:(c + 1) * 128],
                                        xtb16[:, k, :],
                                        start=(k == 0),
                                        stop=(k == KT - 1),
                                        skip_group_check=True,
                                    )
                            for j in range(KT // 2):
                                lhsT = wqall[:, 2 * j:2 * j + 2,
                                             c * 128:(c + 1) * 128]
                                spans = (
                                    ((256, 512), (512, 1024))
                                    if half == 0 else ((0, 512), (512, 1024))
                                )
                                for (o0, o1) in spans:
                                    nc.tensor.matmul(
                                        ps[:, o0:o1],
                                        lhsT,
                                        xtall[:, 2 * j:2 * j + 2,
                                              h0 + o0:h0 + o1],
                                        start=(j == 0 and o0 != 256),
                                        stop=(j == KT // 2 - 1),
                                        perf_mode=DR,
                                        skip_group_check=True,
                                    )
                            # flush deferred tails now that fresh matmuls are
                            # ahead of them in the tensor queue
                            for t_ in tails:
                                t_()
                            tails = [make_tail(c, half, ps)]
                    # warm the PE through the last tail's structural stall
                    # (its sst/rot matmuls wait on the just-finished group's
                    # scalar ops) so phase D doesn't start clock-throttled
                    for _ in range(24):
                        nc.tensor.ldweights(rmat[:])
                    for t_ in tails:
                        t_()

            # ====== Phases B+C+D+E interleaved (rope / v / attn / o_proj) ==
            with ExitStack() as dctx:
                attp = dctx.enter_context(tc.tile_pool(name="attnT", bufs=30))
                rnp = dctx.enter_context(tc.tile_pool(name="rnorm", bufs=2))
                ostage = dctx.enter_context(tc.tile_pool(name="ostage", bufs=2))
                sc_psp = dctx.enter_context(
                    tc.tile_pool(name="sc_ps", bufs=2, space="PSUM")
                )
                ot_psp = dctx.enter_context(
                    tc.tile_pool(name="ot_ps", bufs=1, space="PSUM")
                )
                rs_psp = dctx.enter_context(
                    tc.tile_pool(name="rs_ps", bufs=1, space="PSUM")
                )
                op_psp = dctx.enter_context(
                    tc.tile_pool(name="op_ps", bufs=2, space="PSUM")
                )

                # ---- attention + o_proj, chunk-major ----
                # software-pipelined by one head: exp tiles for head h are
                # produced while head h-1's attn@v / rowsum matmuls consume.
                # Chunks run 3,2,1,0: the first chunk's deep tensor work
                # hides the exp spin-up latency, and the skinny ic=0 chunk
                # lands at the end where chunk-1's o_proj matmuls fill its
                # bubbles.
                chunk_order = [0, 1, 2, 3]
                for ci, ic in enumerate(chunk_order):
                    i0 = ic * 512
                    njt = 4 * ic + 4
                    at_dt = bf if ic == 0 else f8
                    maskx = masksb if ic == 0 else masks

                    def produce(h):
                        # key tiles processed in PAIRS sharing a 2-bank PSUM
                        # tile and a [128,2,512] at tile: one exp covers an
                        # off-diagonal pair; diagonal pairs get 2 ragged
                        # exps.  exp is biased by -ln4 (cancels between
                        # numerator and rowsum) so e4m3 never saturates.
                        ats = {}
                        pair_at = {}
                        npair = njt // 2
                        pair_order = [npair - 2, npair - 1] + list(range(npair - 2))
                        for p in pair_order:
                            at2 = attp.tile(
                                [128, 2, 512], at_dt, tag="at2", bufs=16,
                                name=f"at2_{ic}_{h}_{p}",
                            )
                            pair_at[p] = at2
                            sc2 = sc_psp.tile(
                                [128, 2, 512], mybir.dt.float32,
                                tag="sc2", name=f"sc2_{ic}_{h}_{p}",
                            )
                            diag = False
                            for s in range(2):
                                jt = 2 * p + s
                                t = jt - 4 * ic
                                if t < 0:
                                    nc.tensor.matmul(
                                        sc2[:, s, :],
                                        kfin[:, jt * 128:(jt + 1) * 128],
                                        qfin[:, h, i0:i0 + 512],
                                        start=True,
                                        stop=True,
                                    )
                                else:
                                    diag = True
                                    nc.tensor.matmul(
                                        sc2[:, s, t * 128:512],
                                        kfin[:, jt * 128:(jt + 1) * 128],
                                        qfin[:, h, i0 + t * 128:i0 + 512],
                                        start=True,
                                        stop=True,
                                    )
                                ats[jt] = at2[:, s, :]
                            if not diag:
                                nc.scalar.activation(
                                    at2[:], sc2[:], Exp,
                                    scale=INV_SQRT_HD, bias=nln4[:],
                                )
                            else:
                                for s in range(2):
                                    jt = 2 * p + s
                                    t = jt - 4 * ic
                                    nc.scalar.activation(
                                        at2[:, s, t * 128:512],
                                        sc2[:, s, t * 128:512],
                                        Exp, scale=INV_SQRT_HD, bias=nln4[:],
                                    )
                                    # only the leading 128 cols need the
                                    # triangle mask; beyond that every key in
                                    # this tile is visible
                                    nc.vector.tensor_mul(
                                        at2[:, s, t * 128:t * 128 + 128],
                                        at2[:, s, t * 128:t * 128 + 128],
                                        maskx[:],
                                    )
                        return ats, pair_at

                    def consume(h, ats, pair_at):
                        ot = ot_psp.tile(
                            [128, 512], mybir.dt.float32, tag="ot",
                            name=f"ot_{ic}_{h}",
                        )
                        rs = rs_psp.tile(
                            [128, 512], mybir.dt.float32, tag="rs",
                            name=f"rs_{ic}_{h}",
                        )
                        # off-diagonal pairs (ic>=1): one DoubleRow matmul
                        # each for attn@v and for the rowsum
                        for p in range(2 * ic):
                            nc.tensor.matmul(
                                ot[:],
                                v38[:, 2 * p:2 * p + 2, :],
                                pair_at[p][:],
                                start=(p == 0),
                                stop=False,
                                perf_mode=DR,
                                skip_group_check=True,
                            )
                        if ic == 0:
                            for dt_ in range(4):
                                jt = dt_
                                t = dt_ * 128
                                nc.tensor.matmul(
                                    ot[:, t:],
                                    v3[:, jt, :],
                                    ats[jt][:, t:],
                                    start=(dt_ == 0),
                                    stop=(dt_ == 3),
                                    skip_group_check=True,
                                )
                        else:
                            # diagonal 512x512 block: each PAIR contributes
                            # over its co-valid column range via one DR
                            # matmul; the leading tile of each pair covers
                            # its solo 128-col strip with a plain matmul
                            j0 = 4 * ic
                            nc.tensor.matmul(
                                ot[:, 0:128], v38[:, j0, :],
                                ats[j0][:, 0:128],
                                start=False, stop=False, skip_group_check=True,
                            )
                            nc.tensor.matmul(
                                ot[:, 128:512], v38[:, j0:j0 + 2, :],
                                pair_at[2 * ic][:, :, 128:512],
                                start=False, stop=False,
                                perf_mode=DR, skip_group_check=True,
                            )
                            nc.tensor.matmul(
                                ot[:, 256:384], v38[:, j0 + 2, :],
                                ats[j0 + 2][:, 256:384],
                                start=False, stop=False, skip_group_check=True,
                            )
                            nc.tensor.matmul(
                                ot[:, 384:512], v38[:, j0 + 2:j0 + 4, :],
                                pair_at[2 * ic + 1][:, :, 384:512],
                                start=False, stop=True,
                                perf_mode=DR, skip_group_check=True,
                            )
                        for p in range(2 * ic):
                            nc.tensor.matmul(
                                rs[:],
                                ones8[:],
                                pair_at[p][:],
                                start=(p == 0),
                                stop=False,
                                perf_mode=DR,
                                skip_group_check=True,
                            )
                        if ic == 0:
                            for dt_ in range(4):
                                t = dt_ * 128
                                nc.tensor.matmul(
                                    rs[:, t:],
                                    onesa[:],
                                    ats[dt_][:, t:],
                                    start=(dt_ == 0),
                                    stop=(dt_ == 3),
                                    skip_group_check=True,
                                )
                        else:
                            j0 = 4 * ic
                            nc.tensor.matmul(
                                rs[:, 0:128], ones8[:, 0, :],
                                ats[j0][:, 0:128],
                                start=False, stop=False, skip_group_check=True,
                            )
                            nc.tensor.matmul(
                                rs[:, 128:512], ones8[:],
                                pair_at[2 * ic][:, :, 128:512],
                                start=False, stop=False,
                                perf_mode=DR, skip_group_check=True,
                            )
                            nc.tensor.matmul(
                                rs[:, 256:384], ones8[:, 0, :],
                                ats[j0 + 2][:, 256:384],
                                start=False, stop=False, skip_group_check=True,
                            )
                            nc.tensor.matmul(
                                rs[:, 384:512], ones8[:],
                                pair_at[2 * ic + 1][:, :, 384:512],
                                start=False, stop=True,
                                perf_mode=DR, skip_group_check=True,
                            )
                        rr = rnp.tile([128, 512], mybir.dt.float32, tag="rr")
                        nc.vector.reciprocal_approx_fast(rr[:], rs[:])
                        nc.vector.tensor_mul(otsb[:, h, i0:i0 + 512], ot[:], rr[:])

                    def oproj_m(m):
                        ob = ostage.tile([128, H], bf, tag="ob", name=f"ob{m}")
                        for nn in range(4):
                            op = op_psp.tile(
                                [128, 512], mybir.dt.float32, tag="op",
                                name=f"op{m}_{nn}",
                            )
                            for h in range(GQ):
                                nc.tensor.matmul(
                                    op[:],
                                    otsb[:, h, m * 128:(m + 1) * 128],
                                    wo_sb[:, h, nn * 512:(nn + 1) * 512],
                                    start=(h == 0),
                                    stop=(h == GQ - 1),
                                )
                            if nn % 2 == 0:
                                nc.scalar.copy(
                                    ob[:, nn * 512:(nn + 1) * 512], op[:]
                                )
                            else:
                                nc.vector.tensor_copy(
                                    ob[:, nn * 512:(nn + 1) * 512], op[:]
                                )
                        for nn in range(4):
                            # closing m-tiles: spread the drain DMAs over
                            # four (by then idle) engine queues so issue and
                            # transfer parallelize at the kernel tail
                            eng = (
                                (nc.sync, nc.scalar, nc.gpsimd, nc.sync)[nn]
                                if m >= 12 else nc.sync
                            )
                            eng.dma_start(
                                out=out_d[
                                    m * 128:(m + 1) * 128,
                                    nn * 512:(nn + 1) * 512,
                                ],
                                in_=ob[:, nn * 512:(nn + 1) * 512],
                            )

                    pend = []
                    depth = 1
                    for h in range(GQ):
                        ats, pair_at = produce(h)
                        if ci == 0 and h == 0:
                            # phase-D spin-up is paced by the first exps and
                            # the draining phase-A pipeline: keep the PE warm
                            for _ in range(6):
                                nc.tensor.ldweights(rmat[:])
                        if ci > 0:
                            oproj_m(chunk_order[ci - 1] * 4 + h)
                        pend.append((h, ats, pair_at))
                        if len(pend) > depth:
                            consume(*pend.pop(0))
                    for p_ in pend:
                        consume(*p_)
                    if ci == 3:
                        # the final o_proj's matmuls wait on the last
                        # head's normalize chain; keep the PE warm so the
                        # closing matmuls run at full clock
                        for _ in range(26):
                            nc.tensor.ldweights(rmat[:])
                        for mt in range(4):
                            oproj_m(ic * 4 + mt)

    nc.compile()
    return nc


_PROGRAMS = {}


def _get_program(shared_rope=True):
    if shared_rope not in _PROGRAMS:
        _PROGRAMS[shared_rope] = _build_program(shared_rope)
    return _PROGRAMS[shared_rope]


def _host_consts():
    # rot matrix: out[d', s] = sum_d R[d, d'] t[d, s] = rot(t)[d', s]
    R = np.zeros((128, 128), dtype=np.float32)
    for dp in range(64):
        R[dp + 64, dp] = -1.0
    for dp in range(64, 128):
        R[dp - 64, dp] = 1.0
    ones = np.ones((128, 128), dtype=np.float32)
    ones8 = np.ones((128, 2, 128), dtype=np.float32)
    # mask[p, f] = 1 where key offset p <= query offset f (diagonal block)
    p = np.arange(128)[:, None]
    f = np.arange(128)[None, :]
    mask = (p <= f).astype(np.float32)
    return (
        R.astype(BF16),
        ones.astype(BF16),
        (ones * ALPHA).astype(BF16),
        ones8.astype(FP8),
        np.ascontiguousarray(mask.astype(FP8)),
        np.ascontiguousarray(mask.astype(BF16)),
    )


def kernel(x, sin, cos, Wq, Wk, Wv, Wo, q_norm_w, k_norm_w):
    from concourse.bass_utils import run_bass_kernel_spmd

    qw_ = np.asarray(q_norm_w, dtype=np.float32)
    kw_ = np.asarray(k_norm_w, dtype=np.float32)
    shared_rope = bool(np.array_equal(qw_, kw_))
    nc = _get_program(shared_rope)

    qw = np.asarray(q_norm_w, dtype=np.float32)
    kw = np.asarray(k_norm_w, dtype=np.float32)
    qw_s = np.roll(qw, -64)
    kw_s = np.roll(kw, -64)
    cosT = np.ascontiguousarray(np.asarray(cos, np.float32).T)  # [128, S]
    sinT = np.ascontiguousarray(np.asarray(sin, np.float32).T)
    cosq = (cosT * qw[:, None]).astype(BF16)
    sinq = (sinT * qw_s[:, None]).astype(BF16)
    cosk = (cosT * kw[:, None]).astype(BF16)
    sink = (sinT * kw_s[:, None]).astype(BF16)
    rmat, ones, onesa, ones8, mask, maskb = _host_consts()

    x = np.asarray(x, np.float32) * S_X
    # pack xT k-tile-contiguous: [KT, 128, S] so each k-tile is one DMA
    # with 2KB-contiguous partition lines; bf16 copy of seq cols [0,512)
    # for the exact v path
    xts = []
    xtbs = []
    for b in range(B):
        xt = np.ascontiguousarray(x[b].T.reshape(KT, 128, S))
        xts.append(xt.astype(FP8))
        xtbs.append(np.ascontiguousarray(xt[:, :, 0:256]).astype(BF16))
    Wq = np.asarray(Wq, np.float32) * S_W
    Wk = np.asarray(Wk, np.float32) * S_W
    Wv = np.asarray(Wv, np.float32) * S_W
    Wo = np.asarray(Wo, np.float32)

    in_maps = []
    for core in range(8):
        b, g = divmod(core, 4)
        def pack(w):
            # [(k p), d] -> [p, k, d] SBUF layout
            kt = w.shape[0] // 128
            return np.ascontiguousarray(
                w.reshape(kt, 128, w.shape[1]).transpose(1, 0, 2))
        wq_slice = pack(Wq[:, g * 512:(g + 1) * 512])
        wk_slice = pack(Wk[:, g * 128:(g + 1) * 128])
        wv_slice = pack(Wv[:, g * 128:(g + 1) * 128])
        in_maps.append(
            {
                "xt": xts[b],
                "xtb": xtbs[b],
                "wq": wq_slice.astype(FP8),
                "wqb": wq_slice.astype(BF16),
                "wk": wk_slice.astype(FP8),
                "wkb": wk_slice.astype(BF16),
                "wv": wv_slice.astype(FP8),
                "wvb": wv_slice.astype(BF16),
                "wo": pack(Wo[g * 512:(g + 1) * 512, :]).astype(BF16),
                "cosq": cosq,
                "sinq": sinq,
                "rmat": rmat,
                "ones": ones,
                "onesa": onesa,
                "ones8": ones8,
                "mask": mask,
                "maskb": maskb,
            }
        )

    if not shared_rope:
        for m in in_maps:
            m["cosk"] = cosk
            m["sink"] = sink
    trace = os.environ.get("KERNEL_TRACE", "0") == "1"
    if trace:
        _inject_ntff_hook()
    res = run_bass_kernel_spmd(nc, in_maps, list(range(8)), trace=trace)
    if trace and res.exec_time_ns is not None:
        print(f"HW exec time: {res.exec_time_ns} ns", file=sys.stderr)
        kernel.last_exec_time_ns = res.exec_time_ns

    out = np.zeros((B, S, H), dtype=np.float32)
    for core in range(8):
        b = core // 4
        out[b] += np.asarray(res.results[core]["out"], dtype=np.float32)
    return out


kernel.last_exec_time_ns = None


def _inject_ntff_hook():
    """Recreate antenv.axon_hooks (absent in this image) so
    run_bass_kernel_spmd(trace=True) can capture NTFF profiles."""
    import types
    import contextlib
    import ctypes

    if "antenv.axon_hooks" in sys.modules:
        return
    so_path = "/opt/axon/libaxon_pjrt.so"
    try:
        lib = ctypes.CDLL(so_path)
        lib.axon_start_nrt_profile.argtypes = [
            ctypes.POINTER(ctypes.c_int64),
            ctypes.c_size_t,
        ]
        lib.axon_start_nrt_profile.restype = ctypes.c_int64
        lib.axon_stop_nrt_profile.argtypes = [ctypes.c_char_p]
        lib.axon_stop_nrt_profile.restype = ctypes.c_int64
    except (OSError, AttributeError):
        return

    @contextlib.contextmanager
    def _hook(output_dir, device_ids):
        import jax

        jax.devices()
        if device_ids:
            ids = (ctypes.c_int64 * len(device_ids))(*device_ids)
            rc = lib.axon_start_nrt_profile(ids, len(device_ids))
        else:
            rc = lib.axon_start_nrt_profile(None, 0)
        if rc != 0:
            raise RuntimeError(f"axon_start_nrt_profile rc={rc}")
        try:
            yield
        finally:
            n = lib.axon_stop_nrt_profile(str(output_dir).encode())
            print(f"profile: {n} file(s) -> {output_dir}", file=sys.stderr)

    mod = types.ModuleType("antenv.axon_hooks")
    mod.get_axon_ntff_profile_hook = lambda: _hook
    sys.modules["antenv.axon_hooks"] = mod


# revision 41
# speedup vs baseline: 1.0099x; 1.0099x over previous
"""Self-contained Trainium2 Bass kernel for nn_Attention_59253368816224.

GQA attention block: per-head RMSNorm on Q/K, RoPE, causal softmax
attention, o_proj.  B=2, S=2048, H=2048, 16 Q heads / 4 KV heads,
head_dim=128.

Sharding: 8 cores = 2 batches x 4 KV groups.  Core c -> (b=c//4, g=c%4)
owns 4 Q heads + 1 KV head.  o_proj is row-parallel: each core emits a
partial [S, H] output contracted over its 512 hidden dims; the host sums
the 4 partials per batch in fp32.

fp8 (e4m3) DoubleRow (2x bf16 throughput on 256-deep contractions) is
used ONLY where its ~3.6% per-element quantization noise is attenuated
by softmax averaging (verified vs a host emulation of the full
quantization pipeline):
  - q/k projection (k-tile pairs): noise enters scores only as exp
    jitter, averaged over the effective key count.
  - v projection for seq>=512 and attn@v for query chunks ic>=1: short
    rows (few keys, no averaging) keep an exact bf16 path; long rows
    attenuate.
  - rowsum over off-diagonal key-tile pairs (fp8 ones).
Everything on the direct output path (o_proj, otsb, Wo) stays bf16:
matmul-input noise passes to the output UNattenuated (random-sign
contraction preserves SNR), and a single e4m3 stage there costs ~2.6%
max-rel, more than the whole 2e-2 budget.

All fp8 operands are pre-scaled by powers of two into e4m3's normal
range (subnormals flush on HW): x*8, Wq/Wk/Wv*256 (RMSNorm cancels the
q/k scale; the ic=0 rowsum uses an ALPHA-valued ones matrix to cancel
the v scale), exp biased by -ln4 (max observed score ~5.3 -> exp/4 ~ 49
< e4m3 max).

Device pipeline:
  A) W-stationary QKV projection from host-pre-transposed fp8 xT,
     DoubleRow over k-tile pairs (v cols [0,512) from a parallel bf16
     copy of x), producing transposed qT/kT/vT [d, s]; per-column
     sum-of-squares via selector matmuls -> RMSNorm scale via ACT
     Abs_reciprocal_sqrt (same table set as Square/Copy; frees DVE).
     Scale/rope tails are emitted one proj group late so their sst/rot
     matmuls never stall the in-order tensor queue.
  B) RoPE in the transposed domain: rot(q) via a constant 128x128
     permutation matmul; combine with w-folded cosT/sinT tables.
  C) v transposed back to natural [s, d] with a DMA transpose (bf16,
     ALPHA-scaled), then cast to fp8 v_true with a 2^-11 rescale.
  D) Flash-style causal attention on transposed scores [j, i]: key
     tiles processed in pairs sharing a 2-bank PSUM tile, one exp per
     off-diagonal pair; 128-col triangle masks on DVE.  Chunk ic=0 runs
     fully bf16; chunks ic>=1 write fp8 at tiles, attn@v DoubleRow per
     off-diagonal pair, rowsum DoubleRow per pair (no DVE pre-sums);
     normalizer via DVE fast reciprocal.
  E) o_proj bf16 from otsb against Wo rows (PSUM->SBUF casts split
     scalar/vector), partial output to DRAM bf16.
"""

import os
import sys
import numpy as np
import ml_dtypes

BF16 = ml_dtypes.bfloat16
FP8 = ml_dtypes.float8_e4m3

B = 2
S = 2048
H = 2048
NQH = 16          # total q heads
NKV = 4           # total kv heads
HD = 128          # head dim
GQ = 4            # q heads per core (per kv group)
KT = H // 128     # 16 k-tiles over hidden
ST = S // 128     # 16 s-tiles
RMS_EPS = 1.1920928955078125e-07
INV_SQRT_HD = 1.0 / float(np.sqrt(HD))
LN4 = float(np.log(4.0))

S_X = 8.0          # host scale on x before fp8 quantization
S_W = 256.0        # host scale on Wq/Wk/Wv
ALPHA = S_X * S_W  # scale carried by q/k/v out of the projection (2048)

_PROGRAM = None


def _build_program(shared_rope=True):
    import concourse.bacc as bacc
    import concourse.tile as tile
    from concourse import mybir
    from contextlib import ExitStack

    bf = mybir.dt.bfloat16
    f8 = mybir.dt.float8e4
    DR = mybir.MatmulPerfMode.DoubleRow

    nc = bacc.Bacc("TRN2", target_bir_lowering=False, debug=False, num_devices=8)

    # ---- DRAM I/O (per-core values supplied via in_maps) ----
    xt_d = nc.dram_tensor("xt", (KT, 128, S), f8, kind="ExternalInput")
    xtb_d = nc.dram_tensor("xtb", (KT, 128, 256), bf, kind="ExternalInput")
    # weights arrive pre-packed in SBUF layout [128, k, d]: a plain
    # contiguous DMA costs ~0.6us to issue vs ~3-5us for the gather
    # pattern of an on-the-fly (k p) d -> p k d rearrange
    wq_d = nc.dram_tensor("wq", (128, KT, GQ * HD), f8, kind="ExternalInput")
    wqb_d = nc.dram_tensor("wqb", (128, KT, GQ * HD), bf, kind="ExternalInput")
    wk_d = nc.dram_tensor("wk", (128, KT, HD), f8, kind="ExternalInput")
    wkb_d = nc.dram_tensor("wkb", (128, KT, HD), bf, kind="ExternalInput")
    wv_d = nc.dram_tensor("wv", (128, KT, HD), f8, kind="ExternalInput")
    wvb_d = nc.dram_tensor("wvb", (128, KT, HD), bf, kind="ExternalInput")
    wo_d = nc.dram_tensor("wo", (128, GQ, H), bf, kind="ExternalInput")
    cosq_d = nc.dram_tensor("cosq", (HD, S), bf, kind="ExternalInput")
    sinq_d = nc.dram_tensor("sinq", (HD, S), bf, kind="ExternalInput")
    if not shared_rope:
        cosk_d = nc.dram_tensor("cosk", (HD, S), bf, kind="ExternalInput")
        sink_d = nc.dram_tensor("sink", (HD, S), bf, kind="ExternalInput")
    rmat_d = nc.dram_tensor("rmat", (128, 128), bf, kind="ExternalInput")
    ones_d = nc.dram_tensor("ones", (128, 128), bf, kind="ExternalInput")
    onesa_d = nc.dram_tensor("onesa", (128, 128), bf, kind="ExternalInput")
    ones8_d = nc.dram_tensor("ones8", (128, 2, 128), f8, kind="ExternalInput")
    mask_d = nc.dram_tensor("mask", (128, 128), f8, kind="ExternalInput")
    maskb_d = nc.dram_tensor("maskb", (128, 128), bf, kind="ExternalInput")
    out_d = nc.dram_tensor("out", (S, H), bf, kind="ExternalOutput")
    # internal scratch for the v transpose
    vt_scratch = nc.dram_tensor("vt_scratch", (HD, S), bf)

    Exp = mybir.ActivationFunctionType.Exp
    ARsqrt = mybir.ActivationFunctionType.Abs_reciprocal_sqrt
    Square = mybir.ActivationFunctionType.Square

    with tile.TileContext(nc) as tc:
        with ExitStack() as ctx:
            consts = ctx.enter_context(tc.tile_pool(name="consts", bufs=1))
            persist = ctx.enter_context(tc.tile_pool(name="persist", bufs=1))

            # ---- persistent intermediates ----
            qkvbf = persist.tile([128, 5, S], bf)      # raw transposed q(4)/k
            vt = persist.tile([128, S], bf)            # raw transposed v
            qfin = persist.tile([128, GQ, S], bf)      # roped+normed qT
            kfin = persist.tile([128, S], bf)          # roped+normed kT
            v3 = persist.tile([128, ST, HD], bf)       # v natural (x ALPHA)
            v38 = persist.tile([128, ST, HD], f8)      # v natural fp8 (true)
            otsb = persist.tile([128, GQ, S], bf)      # oT per head
            wo_sb = persist.tile([128, GQ, H], bf)     # o_proj weights

            # ---- constant tiles ----
            cosq = consts.tile([128, S], bf)
            sinq = consts.tile([128, S], bf)
            if shared_rope:
                cosk, sink = cosq, sinq
            else:
                cosk = consts.tile([128, S], bf)
                sink = consts.tile([128, S], bf)
            rmat = consts.tile([128, 128], bf)
            onesm = consts.tile([128, 128], bf)
            onesa = consts.tile([128, 128], bf)
            ones8 = consts.tile([128, 2, 128], f8)
            masks = consts.tile([128, 128], f8)
            masksb = consts.tile([128, 128], bf)
            eps128 = consts.tile([128, 1], mybir.dt.float32)
            nc.vector.memset(eps128[:], RMS_EPS)
            nln4 = consts.tile([128, 1], mybir.dt.float32)
            nc.vector.memset(nln4[:], -LN4)

            # ============ Phase A: QKV projection (+ per-chunk scales) ====
            with tc.tile_pool(name="proj_in", bufs=1) as proj_in, \
                 tc.tile_pool(name="sqp", bufs=2) as sqp, \
                 tc.tile_pool(name="scb", bufs=4) as scbp, \
                 tc.tile_pool(name="ropet", bufs=2) as ropet:
                xtall = proj_in.tile([128, KT, S], f8, tag="xtall", name="xtall")
                xtb16 = proj_in.tile([128, KT, 256], bf, tag="xtb16", name="xtb16")
                wqall = proj_in.tile([128, KT, GQ * HD], f8, tag="wqall", name="wqall")
                wqb16 = proj_in.tile([128, KT, GQ * HD], bf, tag="wqb16", name="wqb16")
                wkall = proj_in.tile([128, KT, HD], f8, tag="wkall", name="wkall")
                wkb16 = proj_in.tile([128, KT, HD], bf, tag="wkb16", name="wkb16")
                wvall = proj_in.tile([128, KT, HD], f8, tag="wvall", name="wvall")
                wvb = proj_in.tile([128, KT, HD], bf, tag="wvb", name="wvb")
                # DMA issue costs ~0.6us per call on an engine's serial
                # queue: spread input issues across THREE queues (sync:
                # fp8 x + fp8 weights; vector: bf16 x + bf16 weights;
                # scalar: tables + o_proj weights) so transfers start early
                # and the q weights aren't stuck behind 16 x-tile issues.
                # the very first matmul (k-proj bf16 chain, k=0) needs only
                # wkb16 + xtb16[0]: put them first so compute starts ~10us
                # earlier; the fp8 DR operands follow immediately after
                nc.sync.dma_start(out=wkb16[:], in_=wkb_d[:])
                nc.sync.dma_start(out=xtb16[:, 0, :], in_=xtb_d[0, :, :])
                nc.sync.dma_start(out=xtb16[:, 1, :], in_=xtb_d[1, :, :])
                nc.sync.dma_start(out=wvb[:], in_=wvb_d[:])
                nc.sync.dma_start(out=wkall[:], in_=wk_d[:])
                nc.sync.dma_start(out=xtall[:, 0, :], in_=xt_d[0, :, :])
                nc.sync.dma_start(out=xtall[:, 1, :], in_=xt_d[1, :, :])
                nc.sync.dma_start(out=wvall[:], in_=wv_d[:])
                for k in range(2, KT):
                    nc.sync.dma_start(out=xtb16[:, k, :], in_=xtb_d[k, :, :])
                    nc.sync.dma_start(out=xtall[:, k, :], in_=xt_d[k, :, :])
                nc.sync.dma_start(out=wqall[:], in_=wq_d[:])
                nc.sync.dma_start(out=wqb16[:], in_=wqb_d[:])
                nc.scalar.dma_start(out=cosq[:], in_=cosq_d[:])
                nc.scalar.dma_start(out=sinq[:], in_=sinq_d[:])
                if not shared_rope:
                    nc.scalar.dma_start(out=cosk[:], in_=cosk_d[:])
                    nc.scalar.dma_start(out=sink[:], in_=sink_d[:])
                nc.scalar.dma_start(out=rmat[:], in_=rmat_d[:])
                nc.scalar.dma_start(out=onesm[:], in_=ones_d[:])
                nc.scalar.dma_start(out=onesa[:], in_=onesa_d[:])
                nc.scalar.dma_start(out=ones8[:], in_=ones8_d[:])
                nc.scalar.dma_start(out=masks[:], in_=mask_d[:])
                nc.scalar.dma_start(out=masksb[:], in_=maskb_d[:])
                # o_proj weights stream during the post-input DMA lull so
                # phase E doesn't wait on them
                nc.scalar.dma_start(out=wo_sb[:], in_=wo_d[:])

                scbs = {}

                def make_tail(c, half, ps):
                    # copy + norm-scale + rope for one finished proj group.
                    # Emitted AFTER the next group's matmuls so the sst/rot
                    # matmuls never stall the in-order tensor queue.
                    def tail():
                        h0 = half * 1024
                        dst = qkvbf[:, c, h0:h0 + 1024]
                        # scale = 1/sqrt(mean+eps) via Abs_reciprocal_sqrt
                        # (same ACT table set as Square/Copy).
                        # Square first: it reads PSUM directly, so the sst
                        # matmul isn't queued behind the copy on scalar
                        sq = sqp.tile([128, 1024], bf, tag="sq", name="sq")
                        nc.scalar.activation(sq[:], ps[:], Square)
                        nc.scalar.copy(dst, ps[:])
                        for nn in range(2):
                            sst = ss_ps.tile(
                                [128, 512], mybir.dt.float32,
                                tag="sst", name=f"sst_{c}_{half}_{nn}",
                            )
                            nc.tensor.matmul(
                                sst[:],
                                onesm[:],
                                sq[:, nn * 512:(nn + 1) * 512],
                                start=True,
                                stop=True,
                            )
                            scb = scbp.tile(
                                [128, 512], mybir.dt.float32,
                                tag="scb", name="scb",
                            )
                            scbs[(c, half * 2 + nn)] = scb
                            nc.scalar.activation(
                                scb[:], sst[:], ARsqrt,
                                bias=eps128[:], scale=1.0 / HD,
                            )
                        cosx = cosq if c < 4 else cosk
                        sinx = sinq if c < 4 else sink
                        for q4 in (2 * half, 2 * half + 1):
                            o0 = q4 * 512
                            scb = scbs[(c, q4)]
                            src_ap = qkvbf[:, c, o0:o0 + 512]
                            rot = rot_psp.tile(
                                [128, 512], mybir.dt.float32,
                                tag="rot", name="rot",
                            )
                            nc.tensor.matmul(
                                rot[:], rmat[:], src_ap, start=True, stop=True
                            )
                            a = ropet.tile([128, 512], bf, tag="a")
                            bb = ropet.tile([128, 512], bf, tag="b")
                            cc = ropet.tile([128, 512], bf, tag="c")
                            nc.gpsimd.tensor_mul(a[:], src_ap, cosx[:, o0:o0 + 512])
                            nc.vector.tensor_mul(bb[:], rot[:], sinx[:, o0:o0 + 512])
                            nc.vector.tensor_add(cc[:], a[:], bb[:])
                            dst2 = (
                                qfin[:, c, o0:o0 + 512]
                                if c < 4
                                else kfin[:, o0:o0 + 512]
                            )
                            nc.vector.tensor_mul(dst2, cc[:], scb[:])
                    return tail

                def make_late_tail(half, sq):
                    # norm-scale + rope for the k chunk, computed from the
                    # squared values after its PSUM pool has closed
                    def tail():
                        for nn in range(2):
                            sst = ss_ps.tile(
                                [128, 512], mybir.dt.float32,
                                tag="sst", name=f"sst_4_{half}_{nn}",
                            )
                            nc.tensor.matmul(
                                sst[:],
                                onesm[:],
                                sq[:, nn * 512:(nn + 1) * 512],
                                start=True,
                                stop=True,
                            )
                            scb = scbp.tile(
                                [128, 512], mybir.dt.float32,
                                tag="scb", name="scb",
                            )
                            scbs[(4, half * 2 + nn)] = scb
                            nc.scalar.activation(
                                scb[:], sst[:], ARsqrt,
                                bias=eps128[:], scale=1.0 / HD,
                            )
                        for q4 in (2 * half, 2 * half + 1):
                            o0 = q4 * 512
                            scb = scbs[(4, q4)]
                            src_ap = qkvbf[:, 4, o0:o0 + 512]
                            rot = rot_psp.tile(
                                [128, 512], mybir.dt.float32,
                                tag="rot", name="rot",
                            )
                            nc.tensor.matmul(
                                rot[:], rmat[:], src_ap, start=True, stop=True
                            )
                            a = ropet.tile([128, 512], bf, tag="a")
                            bb = ropet.tile([128, 512], bf, tag="b")
                            cc = ropet.tile([128, 512], bf, tag="c")
                            nc.gpsimd.tensor_mul(a[:], src_ap, cosk[:, o0:o0 + 512])
                            nc.vector.tensor_mul(bb[:], rot[:], sink[:, o0:o0 + 512])
                            nc.vector.tensor_add(cc[:], a[:], bb[:])
                            nc.vector.tensor_mul(
                                kfin[:, o0:o0 + 512], cc[:], scb[:]
                            )
                    return tail

                # ---- k and v projections jointly while x tiles stream in.
                # k: DoubleRow fp8 over k-tile pairs.  v: bf16 for seq cols
                # [0,512) (exact path for short attention rows), DoubleRow
                # fp8 beyond.  Their 4 accumulator groups take all 8 PSUM
                # banks, so this pool closes before the ss/rot pools open.
                sq4 = []
                with tc.tile_pool(name="ps45", bufs=4, space="PSUM") as ps45p:
                    ps45 = [
                        ps45p.tile(
                            [128, 1024], mybir.dt.float32,
                            tag="ps45", name=f"ps45_{g}",
                        )
                        for g in range(4)
                    ]
                    for j in range(KT // 2):
                        # k proj seq [0,256): bf16 (exact path for short
                        # attention rows), two k-tiles per j
                        for s_ in range(2):
                            k = 2 * j + s_
                            nc.tensor.matmul(
                                ps45[0][:, 0:256],
                                wkb16[:, k, :],
                                xtb16[:, k, :],
                                start=(k == 0),
                                stop=(k == KT - 1),
                                skip_group_check=True,
                            )
                        # k proj seq [256,2048): DR pairs (ragged first)
                        for (g, o0, o1) in ((0, 256, 512), (0, 512, 1024),
                                            (1, 1024, 1536), (1, 1536, 2048)):
                            # the [256:512] region shares a PSUM bank with
                            # the bf16 chain: its k==0 start already zeroed
                            # the whole 2KB zero-region, so never re-start
                            nc.tensor.matmul(
                                ps45[g][:, o0 - g * 1024:o1 - g * 1024],
                                wkall[:, 2 * j:2 * j + 2, :],
                                xtall[:, 2 * j:2 * j + 2, o0:o1],
                                start=(j == 0 and o0 != 256),
                                stop=(j == KT // 2 - 1),
                                perf_mode=DR,
                                skip_group_check=True,
                            )
                        # v proj seq [0,256): bf16, two k-tiles per j
                        for s_ in range(2):
                            k = 2 * j + s_
                            nc.tensor.matmul(
                                ps45[2][:, 0:256],
                                wvb[:, k, :],
                                xtb16[:, k, :],
                                start=(k == 0),
                                stop=(k == KT - 1),
                                skip_group_check=True,
                            )
                        # v proj seq [256,2048): DR pairs (ragged first)
                        for (g, o0, o1) in ((2, 256, 512), (2, 512, 1024),
                                            (3, 1024, 1536), (3, 1536, 2048)):
                            nc.tensor.matmul(
                                ps45[g][:, o0 - (g - 2) * 1024:o1 - (g - 2) * 1024],
                                wvall[:, 2 * j:2 * j + 2, :],
                                xtall[:, 2 * j:2 * j + 2, o0:o1],
                                start=(j == 0 and o0 != 256),
                                stop=(j == KT // 2 - 1),
                                perf_mode=DR,
                                skip_group_check=True,
                            )
                    for half in range(2):
                        sq = sqp.tile([128, 1024], bf, tag="sq", name="sq4")
                        nc.scalar.activation(sq[:], ps45[half][:], Square)
                        nc.scalar.copy(
                            qkvbf[:, 4, half * 1024:(half + 1) * 1024],
                            ps45[half][:],
                        )
                        sq4.append(sq)
                        nc.vector.tensor_copy(
                            vt[:, half * 1024:(half + 1) * 1024],
                            ps45[2 + half][:],
                        )
                    # v transpose DMAs issue on the sync queue, which has
                    # drained its input issues by this point
                    nc.sync.dma_start(out=vt_scratch[:], in_=vt[:])
                    nc.sync.dma_start_transpose(out=v3[:], in_=vt_scratch[:])
                    # fp8 cast removing the ALPHA scale (exact power of 2)
                    nc.vector.tensor_scalar_mul(v38[:], v3[:], 1.0 / ALPHA)

                with tc.tile_pool(name="qkv_ps", bufs=2, space="PSUM") as qkv_ps, \
                     tc.tile_pool(name="ss_ps", bufs=2, space="PSUM") as ss_ps, \
                     tc.tile_pool(name="rot_ps", bufs=2, space="PSUM") as rot_psp:
                    tails = [make_late_tail(0, sq4[0]), make_late_tail(1, sq4[1])]
                    # half=1 groups first: they are pure DoubleRow and need
                    # only wqall, covering the in-flight wqb16 transfer with
                    # ~13us of matmuls before the bf16 chains start
                    for (c, half) in ((0, 1), (1, 1), (2, 1), (3, 1),
                                      (0, 0), (1, 0), (2, 0), (3, 0)):
                        if True:
                            h0 = half * 1024
                            ps = qkv_ps.tile(
                                [128, 1024], mybir.dt.float32, tag="ps", name="ps"
                            )
                            if half == 0:
                                # q proj seq [0,256): bf16 exact path
                                for k in range(KT):
                                    nc.tensor.matmul(
                                        ps[:, 0:256],
                                        wqb16[:, k, c * 128:(c + 1) * 128],
                                        xtb16[:, k, :],
                                        start=(k == 0),
                                        stop=(k == KT - 1),
                                        skip_group_check=True,
                                    )
                            for j in range(KT // 2):
                                lhsT = wqall[:, 2 * j:2 * j + 2,
                                             c * 128:(c + 1) * 128]
                                spans = (
                                    ((256, 512), (512, 1024))
                                    if half == 0 else ((0, 512), (512, 1024))
                                )
                                for (o0, o1) in spans:
                                    nc.tensor.matmul(
                                        ps[:, o0:o1],
                                        lhsT,
                                        xtall[:, 2 * j:2 * j + 2,
                                              h0 + o0:h0 + o1],
                                        start=(j == 0 and o0 != 256),
                                        stop=(j == KT // 2 - 1),
                                        perf_mode=DR,
                                        skip_group_check=True,
                                    )
                            # flush deferred tails now that fresh matmuls are
                            # ahead of them in the tensor queue
                            for t_ in tails:
                                t_()
                            tails = [make_tail(c, half, ps)]
                    # warm the PE through the last tail's structural stall
                    # (its sst/rot matmuls wait on the just-finished group's
                    # scalar ops) so phase D doesn't start clock-throttled
                    for _ in range(24):
                        nc.tensor.ldweights(rmat[:])
                    for t_ in tails:
                        t_()

            # ====== Phases B+C+D+E interleaved (rope / v / attn / o_proj) ==
            with ExitStack() as dctx:
                attp = dctx.enter_context(tc.tile_pool(name="attnT", bufs=30))
                rnp = dctx.enter_context(tc.tile_pool(name="rnorm", bufs=2))
                ostage = dctx.enter_context(tc.tile_pool(name="ostage", bufs=2))
                sc_psp = dctx.enter_context(
                    tc.tile_pool(name="sc_ps", bufs=2, space="PSUM")
                )
                ot_psp = dctx.enter_context(
                    tc.tile_pool(name="ot_ps", bufs=1, space="PSUM")
                )
                rs_psp = dctx.enter_context(
                    tc.tile_pool(name="rs_ps", bufs=1, space="PSUM")
                )
                op_psp = dctx.enter_context(
                    tc.tile_pool(name="op_ps", bufs=2, space="PSUM")
                )

                # ---- attention + o_proj, chunk-major ----
                # software-pipelined by one head: exp tiles for head h are
                # produced while head h-1's attn@v / rowsum matmuls consume.
                # Chunks run 3,2,1,0: the first chunk's deep tensor work
                # hides the exp spin-up latency, and the skinny ic=0 chunk
                # lands at the end where chunk-1's o_proj matmuls fill its
                # bubbles.
                chunk_order = [0, 1, 2, 3]
                for ci, ic in enumerate(chunk_order):
                    i0 = ic * 512
                    njt = 4 * ic + 4
                    at_dt = bf if ic == 0 else f8
                    maskx = masksb if ic == 0 else masks

                    def produce(h):
                        # key tiles processed in PAIRS sharing a 2-bank PSUM
                        # tile and a [128,2,512] at tile: one exp covers an
                        # off-diagonal pair; diagonal pairs get 2 ragged
                        # exps.  exp is biased by -ln4 (cancels between
                        # numerator and rowsum) so e4m3 never saturates.
                        ats = {}
                        pair_at = {}
                        npair = njt // 2
                        pair_order = [npair - 2, npair - 1] + list(range(npair - 2))
                        for p in pair_order:
                            at2 = attp.tile(
                                [128, 2, 512], at_dt, tag="at2", bufs=16,
                                name=f"at2_{ic}_{h}_{p}",
                            )
                            pair_at[p] = at2
                            sc2 = sc_psp.tile(
                                [128, 2, 512], mybir.dt.float32,
                                tag="sc2", name=f"sc2_{ic}_{h}_{p}",
                            )
                            diag = False
                            for s in range(2):
                                jt = 2 * p + s
                                t = jt - 4 * ic
                                if t < 0:
                                    nc.tensor.matmul(
                                        sc2[:, s, :],
                                        kfin[:, jt * 128:(jt + 1) * 128],
                                        qfin[:, h, i0:i0 + 512],
                                        start=True,
                                        stop=True,
                                    )
                                else:
                                    diag = True
                                    nc.tensor.matmul(
                                        sc2[:, s, t * 128:512],
                                        kfin[:, jt * 128:(jt + 1) * 128],
                                        qfin[:, h, i0 + t * 128:i0 + 512],
                                        start=True,
                                        stop=True,
                                    )
                                ats[jt] = at2[:, s, :]
                            if not diag:
                                nc.scalar.activation(
                                    at2[:], sc2[:], Exp,
                                    scale=INV_SQRT_HD, bias=nln4[:],
                                )
                            else:
                                for s in range(2):
                                    jt = 2 * p + s
                                    t = jt - 4 * ic
                                    nc.scalar.activation(
                                        at2[:, s, t * 128:512],
                                        sc2[:, s, t * 128:512],
                                        Exp, scale=INV_SQRT_HD, bias=nln4[:],
                                    )
                                    # only the leading 128 cols need the
                                    # triangle mask; beyond that every key in
                                    # this tile is visible
                                    nc.vector.tensor_mul(
                                        at2[:, s, t * 128:t * 128 + 128],
                                        at2[:, s, t * 128:t * 128 + 128],
                                        maskx[:],
                                    )
                        return ats, pair_at

                    def consume(h, ats, pair_at):
                        ot = ot_psp.tile(
                            [128, 512], mybir.dt.float32, tag="ot",
                            name=f"ot_{ic}_{h}",
                        )
                        rs = rs_psp.tile(
                            [128, 512], mybir.dt.float32, tag="rs",
                            name=f"rs_{ic}_{h}",
                        )
                        # off-diagonal pairs (ic>=1): one DoubleRow matmul
                        # each for attn@v and for the rowsum
                        for p in range(2 * ic):
                            nc.tensor.matmul(
                                ot[:],
                                v38[:, 2 * p:2 * p + 2, :],
                                pair_at[p][:],
                                start=(p == 0),
                                stop=False,
                                perf_mode=DR,
                                skip_group_check=True,
                            )
                        if ic == 0:
                            for dt_ in range(4):
                                jt = dt_
                                t = dt_ * 128
                                nc.tensor.matmul(
                                    ot[:, t:],
                                    v3[:, jt, :],
                                    ats[jt][:, t:],
                                    start=(dt_ == 0),
                                    stop=(dt_ == 3),
                                    skip_group_check=True,
                                )
                        else:
                            # diagonal 512x512 block: each PAIR contributes
                            # over its co-valid column range via one DR
                            # matmul; the leading tile of each pair covers
                            # its solo 128-col strip with a plain matmul
                            j0 = 4 * ic
                            nc.tensor.matmul(
                                ot[:, 0:128], v38[:, j0, :],
                                ats[j0][:, 0:128],
                                start=False, stop=False, skip_group_check=True,
                            )
                            nc.tensor.matmul(
                                ot[:, 128:512], v38[:, j0:j0 + 2, :],
                                pair_at[2 * ic][:, :, 128:512],
                                start=False, stop=False,
                                perf_mode=DR, skip_group_check=True,
                            )
                            nc.tensor.matmul(
                                ot[:, 256:384], v38[:, j0 + 2, :],
                                ats[j0 + 2][:, 256:384],
                                start=False, stop=False, skip_group_check=True,
                            )
                            nc.tensor.matmul(
                                ot[:, 384:512], v38[:, j0 + 2:j0 + 4, :],
                                pair_at[2 * ic + 1][:, :, 384:512],
                                start=False, stop=True,
                                perf_mode=DR, skip_group_check=True,
                            )
                        for p in range(2 * ic):
                            nc.tensor.matmul(
                                rs[:],
                                ones8[:],
                                pair_at[p][:],
                                start=(p == 0),
                                stop=False,
                                perf_mode=DR,
                                skip_group_check=True,
                            )
                        if ic == 0:
                            for dt_ in range(4):
                                t = dt_ * 128
                                nc.tensor.matmul(
                                    rs[:, t:],
                                    onesa[:],
                                    ats[dt_][:, t:],
                                    start=(dt_ == 0),
                                    stop=(dt_ == 3),
                                    skip_group_check=True,
                                )
                        else:
                            j0 = 4 * ic
                            nc.tensor.matmul(
                                rs[:, 0:128], ones8[:, 0, :],
                                ats[j0][:, 0:128],
                                start=False, stop=False, skip_group_check=True,
                            )
                            nc.tensor.matmul(
                                rs[:, 128:512], ones8[:],
                                pair_at[2 * ic][:, :, 128:512],
                                start=False, stop=False,
                                perf_mode=DR, skip_group_check=True,
                            )
                            nc.tensor.matmul(
                                rs[:, 256:384], ones8[:, 0, :],
                                ats[j0 + 2][:, 256:384],
                                start=False, stop=False, skip_group_check=True,
                            )
                            nc.tensor.matmul(
                                rs[:, 384:512], ones8[:],
                                pair_at[2 * ic + 1][:, :, 384:512],
                                start=False, stop=True,
                                perf_mode=DR, skip_group_check=True,
                            )
                        rr = rnp.tile([128, 512], mybir.dt.float32, tag="rr")
                        nc.vector.reciprocal_approx_fast(rr[:], rs[:])
                        nc.vector.tensor_mul(otsb[:, h, i0:i0 + 512], ot[:], rr[:])

                    def oproj_m(m):
                        ob = ostage.tile([128, H], bf, tag="ob", name=f"ob{m}")
                        for nn in range(4):
                            op = op_psp.tile(
                                [128, 512], mybir.dt.float32, tag="op",
                                name=f"op{m}_{nn}",
                            )
                            for h in range(GQ):
                                nc.tensor.matmul(
                                    op[:],
                                    otsb[:, h, m * 128:(m + 1) * 128],
                                    wo_sb[:, h, nn * 512:(nn + 1) * 512],
                                    start=(h == 0),
                                    stop=(h == GQ - 1),
                                )
                            if nn == 0:
                                # scalar takes only 1 of 4 copies: its queue
                                # is dominated by exps, and op_psp recycling
                                # stalls the o_proj matmuls when copies lag
                                nc.scalar.copy(
                                    ob[:, nn * 512:(nn + 1) * 512], op[:]
                                )
                            else:
                                nc.vector.tensor_copy(
                                    ob[:, nn * 512:(nn + 1) * 512], op[:]
                                )
                        for nn in range(4):
                            # closing m-tiles: spread the drain DMAs over
                            # four (by then idle) engine queues so issue and
                            # transfer parallelize at the kernel tail
                            eng = (
                                (nc.sync, nc.scalar, nc.gpsimd, nc.sync)[nn]
                                if m >= 12 else nc.sync
                            )
                            eng.dma_start(
                                out=out_d[
                                    m * 128:(m + 1) * 128,
                                    nn * 512:(nn + 1) * 512,
                                ],
                                in_=ob[:, nn * 512:(nn + 1) * 512],
                            )

                    pend = []
                    depth = 1
                    for h in range(GQ):
                        ats, pair_at = produce(h)
                        if ci == 0 and h == 0:
                            # phase-D spin-up is paced by the first exps and
                            # the draining phase-A pipeline: keep the PE warm
                            for _ in range(6):
                                nc.tensor.ldweights(rmat[:])
                        if ci > 0:
                            oproj_m(chunk_order[ci - 1] * 4 + h)
                        pend.append((h, ats, pair_at))
                        if len(pend) > depth:
                            consume(*pend.pop(0))
                    for p_ in pend:
                        consume(*p_)
                    if ci == 3:
                        # the final o_proj's matmuls wait on the last
                        # head's normalize chain; keep the PE warm so the
                        # closing matmuls run at full clock
                        for _ in range(26):
                            nc.tensor.ldweights(rmat[:])
                        for mt in range(4):
                            oproj_m(ic * 4 + mt)

    nc.compile()
    return nc


_PROGRAMS = {}


def _get_program(shared_rope=True):
    if shared_rope not in _PROGRAMS:
        _PROGRAMS[shared_rope] = _build_program(shared_rope)
    return _PROGRAMS[shared_rope]


def _host_consts():
    # rot matrix: out[d', s] = sum_d R[d, d'] t[d, s] = rot(t)[d', s]
    R = np.zeros((128, 128), dtype=np.float32)
    for dp in range(64):
        R[dp + 64, dp] = -1.0
    for dp in range(64, 128):
        R[dp - 64, dp] = 1.0
    ones = np.ones((128, 128), dtype=np.float32)
    ones8 = np.ones((128, 2, 128), dtype=np.float32)
    # mask[p, f] = 1 where key offset p <= query offset f (diagonal block)
    p = np.arange(128)[:, None]
    f = np.arange(128)[None, :]
    mask = (p <= f).astype(np.float32)
    return (
        R.astype(BF16),
        ones.astype(BF16),
        (ones * ALPHA).astype(BF16),
        ones8.astype(FP8),
        np.ascontiguousarray(mask.astype(FP8)),
        np.ascontiguousarray(mask.astype(BF16)),
    )


def kernel(x, sin, cos, Wq, Wk, Wv, Wo, q_norm_w, k_norm_w):
    from concourse.bass_utils import run_bass_kernel_spmd

    qw_ = np.asarray(q_norm_w, dtype=np.float32)
    kw_ = np.asarray(k_norm_w, dtype=np.float32)
    shared_rope = bool(np.array_equal(qw_, kw_))
    nc = _get_program(shared_rope)

    qw = np.asarray(q_norm_w, dtype=np.float32)
    kw = np.asarray(k_norm_w, dtype=np.float32)
    qw_s = np.roll(qw, -64)
    kw_s = np.roll(kw, -64)
    cosT = np.ascontiguousarray(np.asarray(cos, np.float32).T)  # [128, S]
    sinT = np.ascontiguousarray(np.asarray(sin, np.float32).T)
    cosq = (cosT * qw[:, None]).astype(BF16)
    sinq = (sinT * qw_s[:, None]).astype(BF16)
    cosk = (cosT * kw[:, None]).astype(BF16)
    sink = (sinT * kw_s[:, None]).astype(BF16)
    rmat, ones, onesa, ones8, mask, maskb = _host_consts()

    x = np.asarray(x, np.float32) * S_X
    # pack xT k-tile-contiguous: [KT, 128, S] so each k-tile is one DMA
    # with 2KB-contiguous partition lines; bf16 copy of seq cols [0,512)
    # for the exact v path
    xts = []
    xtbs = []
    for b in range(B):
        xt = np.ascontiguousarray(x[b].T.reshape(KT, 128, S))
        xts.append(xt.astype(FP8))
        xtbs.append(np.ascontiguousarray(xt[:, :, 0:256]).astype(BF16))
    Wq = np.asarray(Wq, np.float32) * S_W
    Wk = np.asarray(Wk, np.float32) * S_W
    Wv = np.asarray(Wv, np.float32) * S_W
    Wo = np.asarray(Wo, np.float32)

    in_maps = []
    for core in range(8):
        b, g = divmod(core, 4)
        def pack(w):
            # [(k p), d] -> [p, k, d] SBUF layout
            kt = w.shape[0] // 128
            return np.ascontiguousarray(
                w.reshape(kt, 128, w.shape[1]).transpose(1, 0, 2))
        wq_slice = pack(Wq[:, g * 512:(g + 1) * 512])
        wk_slice = pack(Wk[:, g * 128:(g + 1) * 128])
        wv_slice = pack(Wv[:, g * 128:(g + 1) * 128])
        in_maps.append(
            {
                "xt": xts[b],
                "xtb": xtbs[b],
                "wq": wq_slice.astype(FP8),
                "wqb": wq_slice.astype(BF16),
                "wk": wk_slice.astype(FP8),
                "wkb": wk_slice.astype(BF16),
                "wv": wv_slice.astype(FP8),
                "wvb": wv_slice.astype(BF16),
                "wo": pack(Wo[g * 512:(g + 1) * 512, :]).astype(BF16),
                "cosq": cosq,
                "sinq": sinq,
                "rmat": rmat,
                "ones": ones,
                "onesa": onesa,
                "ones8": ones8,
                "mask": mask,
                "maskb": maskb,
            }
        )

    if not shared_rope:
        for m in in_maps:
            m["cosk"] = cosk
            m["sink"] = sink
    trace = os.environ.get("KERNEL_TRACE", "0") == "1"
    if trace:
        _inject_ntff_hook()
    res = run_bass_kernel_spmd(nc, in_maps, list(range(8)), trace=trace)
    if trace and res.exec_time_ns is not None:
        print(f"HW exec time: {res.exec_time_ns} ns", file=sys.stderr)
        kernel.last_exec_time_ns = res.exec_time_ns

    out = np.zeros((B, S, H), dtype=np.float32)
    for core in range(8):
        b = core // 4
        out[b] += np.asarray(res.results[core]["out"], dtype=np.float32)
    return out


kernel.last_exec_time_ns = None


def _inject_ntff_hook():
    """Recreate antenv.axon_hooks (absent in this image) so
    run_bass_kernel_spmd(trace=True) can capture NTFF profiles."""
    import types
    import contextlib
    import ctypes

    if "antenv.axon_hooks" in sys.modules:
        return
    so_path = "/opt/axon/libaxon_pjrt.so"
    try:
        lib = ctypes.CDLL(so_path)
        lib.axon_start_nrt_profile.argtypes = [
            ctypes.POINTER(ctypes.c_int64),
            ctypes.c_size_t,
        ]
        lib.axon_start_nrt_profile.restype = ctypes.c_int64
        lib.axon_stop_nrt_profile.argtypes = [ctypes.c_char_p]
        lib.axon_stop_nrt_profile.restype = ctypes.c_int64
    except (OSError, AttributeError):
        return

    @contextlib.contextmanager
    def _hook(output_dir, device_ids):
        import jax

        jax.devices()
        if device_ids:
            ids = (ctypes.c_int64 * len(device_ids))(*device_ids)
            rc = lib.axon_start_nrt_profile(ids, len(device_ids))
        else:
            rc = lib.axon_start_nrt_profile(None, 0)
        if rc != 0:
            raise RuntimeError(f"axon_start_nrt_profile rc={rc}")
        try:
            yield
        finally:
            n = lib.axon_stop_nrt_profile(str(output_dir).encode())
            print(f"profile: {n} file(s) -> {output_dir}", file=sys.stderr)

    mod = types.ModuleType("antenv.axon_hooks")
    mod.get_axon_ntff_profile_hook = lambda: _hook
    sys.modules["antenv.axon_hooks"] = mod
